# revision 1
# baseline (speedup 1.0000x reference)
"""DeltaNet forward kernel for 8 Trainium2 NeuronCores.

Problem (hardcoded from the task spec): hidden_states [B=4, T=2048, D=1024],
H=4 heads, Dh=256, causal depthwise conv K=4 + silu on q/k/v projections,
q/k l2-normalized per head (q scaled Dh^-0.5), delta-rule recurrence over T,
per-head RMSNorm, merge heads, out = o @ Wo.

Sharding: data-parallel over (batch, head-group): core c -> batch c//2,
head group c%2 (projection columns [512*(c%2), 512*(c%2)+512)). Each core
computes a partial product against its 512 rows of Wo; the host sums the two
partials per batch (the unshard step for the row-parallel output matmul).

Device algorithm: chunked WY form of the delta rule, chunk C=128.
Per chunk (per head): KK = K K^T; A/B = strict lower/upper mask of KK;
R = (I+B)^{-1} = (I-B)(I+B^2)(I+B^4)...(I+B^64) via masked doubling
(B nilpotent); U' = R^T (V - K S); O = Q S + triu(K Q^T)^T U'; S += K^T U'.
S accumulates in PSUM in f32; matmul operands are bf16.
"""

import numpy as np

B, T, D = 4, 2048, 1024
H = 4
DH = D // H          # 256
CONV_K = 4
EPS = 1e-5
NCORES = 8
CG = 512             # columns per core (2 heads)
C = 128              # recurrence chunk length
NCHUNK = T // C      # 16
PAD = 4              # front zero padding on time axis for the causal conv
TOKB = 512           # token block (matmul moving size)
KT = D // 128        # 8 contraction tiles
CT = CG // 128       # 4 column tiles per core
NB = T // TOKB       # 4 token blocks

_CACHE = {}
SILU_NATIVE = True  # CoreSim lacks Silu; set False for simulation runs
DEBUG_SKIP_WO = False  # debug: skip output projection phase


def _build_bass():
    import concourse.bass as bass  # noqa: F401
    import concourse.bacc as bacc
    import concourse.mybir as mybir
    import concourse.tile as tile

    dt = mybir.dt
    nc = bacc.Bacc("TRN2", target_bir_lowering=False, debug=False)

    xT = nc.dram_tensor("xT", [D, T], dt.float16, kind="ExternalInput")
    wq = nc.dram_tensor("wq", [D, CG], dt.float16, kind="ExternalInput")
    wk = nc.dram_tensor("wk", [D, CG], dt.float16, kind="ExternalInput")
    wv = nc.dram_tensor("wv", [D, CG], dt.float16, kind="ExternalInput")
    wo = nc.dram_tensor("wo", [CG, D], dt.float16, kind="ExternalInput")
    cw = nc.dram_tensor("cw", [CG, 3 * CONV_K], dt.float32, kind="ExternalInput")
    consts = nc.dram_tensor("consts", [128, 6 * 128], dt.float16,
                            kind="ExternalInput")
    out = nc.dram_tensor("out", [T, D], dt.float32, kind="ExternalOutput")

    with tile.TileContext(nc) as tc:
        _body(nc, tc, mybir, xT, wq, wk, wv, wo, cw, consts, out)

    nc.compile()
    return nc


def _body(nc, tc, mybir, xT, wq, wk, wv, wo, cw, consts, out):
    dt = mybir.dt
    AF = mybir.ActivationFunctionType
    ALU = mybir.AluOpType
    fp32 = dt.float32
    bf16 = dt.float16  # 16-bit working dtype (fp16: 11-bit mantissa)
    NT = T + PAD

    xT_t = xT.ap().rearrange("(n p) t -> n p t", p=128)       # [8,128,T]
    w_t = {"q": wq.ap().rearrange("(n p) c -> n p c", p=128),
           "k": wk.ap().rearrange("(n p) c -> n p c", p=128),
           "v": wv.ap().rearrange("(n p) c -> n p c", p=128)}
    wo_t = wo.ap().rearrange("(n p) c -> n p c", p=128)       # [4,128,D]
    cw_t = cw.ap().rearrange("(n p) c -> n p c", p=128)       # [4,128,12]
    out_t = out.ap().rearrange("(n p) c -> n p c", p=128)     # [16,128,D]

    # ---------- persistent pool (lives for the whole kernel) ----------
    with tc.tile_pool(name="persist", bufs=1) as persist, \
         tc.tile_pool(name="qkvp", bufs=3 * CT) as qkvp, \
         tc.tile_pool(name="otp", bufs=CT) as otp, \
         tc.tile_pool(name="psw", bufs=6, space="PSUM") as psw, \
         tc.tile_pool(name="pss", bufs=1, space="PSUM") as pss:

        cons = persist.tile([128, 6 * 128], bf16, name="cons", tag="cons")
        nc.sync.dma_start(cons[:], consts.ap())
        ident = cons[:, 0:128]          # identity
        m_bdl = cons[:, 128:256]        # block-diag(32) strict lower, +1
        m_bdu = cons[:, 256:384]        # block-diag(32) strict upper, +1
        m_bdln = cons[:, 384:512]       # block-diag(32) strict lower, -1
        m_fneg = cons[:, 512:640]       # strict upper outside blocks, -1
        m_triuI = cons[:, 640:768]      # i<=j, +1
        ones_col = cons[:, 767:768]     # last col of (i<=j) mask == all ones

        biases = persist.tile([128, 3], dt.float32, name="biases", tag="biases")
        nc.vector.memset(biases[:, 0:1], 1e-6)
        nc.vector.memset(biases[:, 1:2], EPS)
        nc.vector.memset(biases[:, 2:3], 1e-6 * DH)

        cwt = []
        for ct in range(CT):
            t_ = persist.tile([128, 3 * CONV_K], fp32, name=f"cw{ct}",
                              tag=f"cw{ct}")
            nc.sync.dma_start(t_[:], cw_t[ct])
            cwt.append(t_)

        qh, kh, vh = [], [], []
        for lst, nm in ((qh, "q"), (kh, "k"), (vh, "v")):
            for ct in range(CT):
                lst.append(qkvp.tile([128, T], bf16, name=f"{nm}hat{ct}",
                                     tag="qkv"))
        oT = [otp.tile([128, T], bf16, name=f"oT{ct}", tag="oT")
              for ct in range(CT)]

        # ================= phase A: projections + conv + silu + l2norm ====
        with tc.tile_pool(name="xp", bufs=KT) as xp, \
             tc.tile_pool(name="wp", bufs=3 * KT) as wp, \
             tc.tile_pool(name="rawp", bufs=2) as rawp, \
             tc.tile_pool(name="sqp", bufs=4) as sqp, \
             tc.tile_pool(name="stp", bufs=1) as stp, \
             tc.tile_pool(name="bcp", bufs=2) as bcp:

            xt = []
            for kt in range(KT):
                t_ = xp.tile([128, T], bf16, name=f"xt{kt}", tag="xt")
                nc.sync.dma_start(t_[:], xT_t[kt])
                xt.append(t_)
            ws = {}
            for nm in ("q", "k", "v"):
                ws[nm] = []
                for kt in range(KT):
                    t_ = wp.tile([128, CG], bf16, name=f"w{nm}{kt}", tag="w")
                    nc.sync.dma_start(t_[:], w_t[nm][kt])
                    ws[nm].append(t_)

            for ti, (nm, dest) in enumerate((("q", qh), ("k", kh), ("v", vh))):
                sq_tiles = []
                for ct in range(CT):
                    rawt = rawp.tile([128, NT], bf16, name=f"raw{nm}{ct}",
                                     tag="raw")
                    nc.vector.memset(rawt[:, 0:PAD], 0.0)
                    dst = dest[ct]
                    for nb in range(NB):
                        pt = psw.tile([128, TOKB], fp32, name=f"pp{nm}{ct}{nb}",
                                      tag="w")
                        for kt in range(KT):
                            nc.tensor.matmul(
                                pt[:], ws[nm][kt][:, ct * 128:(ct + 1) * 128],
                                xt[kt][:, nb * TOKB:(nb + 1) * TOKB],
                                start=(kt == 0), stop=(kt == KT - 1))
                        nc.scalar.copy(
                            rawt[:, PAD + nb * TOKB:PAD + (nb + 1) * TOKB],
                            pt[:])
                    # causal depthwise conv along t
                    w0 = cwt[ct][:, ti * CONV_K:ti * CONV_K + 1]
                    nc.vector.tensor_scalar_mul(dst[:], rawt[:, 1:1 + T], w0)
                    for i in range(1, CONV_K):
                        wi = cwt[ct][:, ti * CONV_K + i:ti * CONV_K + i + 1]
                        nc.vector.scalar_tensor_tensor(
                            dst[:], rawt[:, 1 + i:1 + i + T], wi, dst[:],
                            ALU.mult, ALU.add)
                    if SILU_NATIVE:
                        nc.scalar.activation(dst[:], dst[:], AF.Silu)
                    else:
                        sg = rawp.tile([128, T], bf16, name=f"sg{nm}{ct}",
                                       tag="raw")
                        nc.scalar.activation(sg[:], dst[:], AF.Sigmoid)
                        nc.vector.tensor_mul(dst[:], dst[:], sg[:])
                    if ti < 2:
                        sqt = sqp.tile([128, T], bf16, name=f"sq{nm}{ct}",
                                       tag="sq")
                        nc.scalar.activation(sqt[:], dst[:], AF.Square)
                        sq_tiles.append(sqt)
                if ti < 2:
                    # per-head l2norm: sumsq rows via ones-matmul, broadcast
                    # to 128 partitions, rsq = scale/sqrt(ss + 1e-6), apply.
                    for head in range(2):
                        bcf = bcp.tile([128, T], fp32, name=f"bcf{nm}{head}",
                                       tag="bcf")
                        for nb in range(NB):
                            prow = psw.tile([1, TOKB], fp32,
                                            name=f"pr{nm}{head}{nb}", tag="w")
                            for cth in range(2):
                                nc.tensor.matmul(
                                    prow[:], ones_col,
                                    sq_tiles[head * 2 + cth][
                                        :, nb * TOKB:(nb + 1) * TOKB],
                                    start=(cth == 0), stop=(cth == 1))
                            rowb = stp.tile([1, TOKB], fp32,
                                            name=f"rb{nm}{head}{nb}",
                                            tag="rowb", bufs=3)
                            nc.scalar.copy(rowb[:], prow[:])
                            nc.gpsimd.partition_broadcast(
                                bcf[:, nb * TOKB:(nb + 1) * TOKB], rowb[:])
                        if ti == 0:
                            # fold Dh^-0.5: 1/(16 sqrt(ss+eps)) =
                            # 1/sqrt(256 ss + 256 eps)
                            nc.scalar.activation(bcf[:], bcf[:], AF.Sqrt,
                                                 bias=biases[:, 2:3],
                                                 scale=float(DH))
                        else:
                            nc.scalar.activation(bcf[:], bcf[:], AF.Sqrt,
                                                 bias=biases[:, 0:1])
                        nc.vector.reciprocal(bcf[:], bcf[:])
                        bcb = bcp.tile([128, T], bf16, name=f"bcb{nm}{head}",
                                       tag="bcb")
                        nc.scalar.copy(bcb[:], bcf[:])
                        for cth in range(2):
                            ct = head * 2 + cth
                            nc.vector.tensor_mul(dest[ct][:], dest[ct][:],
                                                 bcb[:])

        # ================= phase B: delta-rule recurrence =================
        with tc.tile_pool(name="recp", bufs=4) as recp, \
             tc.tile_pool(name="recs", bufs=1) as recs:
            for head in range(2):
                ct0 = head * 2
                s_ps = pss.tile([128, 512], fp32, name=f"sps{head}", tag="sps")
                s_sb = recs.tile([128, 512], bf16, name=f"ssb{head}", tag="ssb",
                                 bufs=2)
                nc.vector.memset(s_sb[:], 0.0)
                for ch in range(NCHUNK):
                    t0 = ch * C
                    QT = [qh[ct0][:, t0:t0 + C], qh[ct0 + 1][:, t0:t0 + C]]
                    KTt = [kh[ct0][:, t0:t0 + C], kh[ct0 + 1][:, t0:t0 + C]]
                    VT = [vh[ct0][:, t0:t0 + C], vh[ct0 + 1][:, t0:t0 + C]]

                    # K, V in [C, Dh] layout via PE transpose (bf16 psum)
                    kcd = recp.tile([128, 256], bf16, name=f"kcd{head}{ch}",
                                    tag="kcd")
                    vcd = recp.tile([128, 256], bf16, name=f"vcd{head}{ch}",
                                    tag="vcd")
                    for i in range(2):
                        ptk = psw.tile([128, 128], bf16, name=f"ptk{head}{ch}{i}",
                                       tag="w")
                        nc.tensor.transpose(ptk[:], KTt[i], ident)
                        nc.scalar.copy(kcd[:, i * 128:(i + 1) * 128], ptk[:])
                        ptv = psw.tile([128, 128], bf16, name=f"ptv{head}{ch}{i}",
                                       tag="w")
                        nc.tensor.transpose(ptv[:], VT[i], ident)
                        nc.scalar.copy(vcd[:, i * 128:(i + 1) * 128], ptv[:])

                    # KK^T; A, B, -A masks
                    pkk = psw.tile([128, 128], fp32, name=f"pkk{head}{ch}",
                                   tag="w")
                    for i in range(2):
                        nc.tensor.matmul(pkk[:], KTt[i], KTt[i], start=(i == 0),
                                         stop=(i == 1))
                    Nl = recp.tile([128, 128], bf16, name=f"Nl{head}{ch}",
                                   tag="Nl")
                    Nln = recp.tile([128, 128], bf16, name=f"Nln{head}{ch}",
                                    tag="Nln")
                    Nu = recp.tile([128, 128], bf16, name=f"Nu{head}{ch}",
                                   tag="Nu")
                    FnT = recp.tile([128, 128], bf16, name=f"FnT{head}{ch}",
                                    tag="FnT")
                    nc.vector.tensor_mul(Nl[:], pkk[:], m_bdl)
                    nc.vector.tensor_mul(Nu[:], pkk[:], m_bdu)
                    nc.vector.tensor_mul(Nln[:], pkk[:], m_bdln)
                    nc.vector.tensor_mul(FnT[:], pkk[:], m_fneg)

                    # R = D^T = (I+Nu)^{-1}, block-diag(32): 4 exact levels
                    pR = psw.tile([128, 128], fp32, name=f"pR{head}{ch}",
                                  tag="w")
                    nc.tensor.matmul(pR[:], ident, ident, start=True,
                                     stop=False)
                    nc.tensor.matmul(pR[:], Nln[:], ident, start=False,
                                     stop=True)
                    Rm = recp.tile([128, 128], bf16, name=f"Rm{head}{ch}0",
                                   tag="Rm")
                    nc.scalar.copy(Rm[:], pR[:])
                    Pm, Qm = Nl, Nu
                    for lvl in range(3):
                        pp = psw.tile([128, 128], fp32,
                                      name=f"pp{head}{ch}{lvl}", tag="w")
                        nc.tensor.matmul(pp[:], Qm[:], Pm[:], start=True,
                                         stop=True)
                        Pn = recp.tile([128, 128], bf16,
                                       name=f"Pn{head}{ch}{lvl}", tag="Pn")
                        nc.scalar.copy(Pn[:], pp[:])
                        if lvl < 2:
                            pq = psw.tile([128, 128], fp32,
                                          name=f"pq{head}{ch}{lvl}", tag="w")
                            nc.tensor.matmul(pq[:], Pm[:], Qm[:], start=True,
                                             stop=True)
                            Qn = recp.tile([128, 128], bf16,
                                           name=f"Qn{head}{ch}{lvl}", tag="Qn")
                            nc.scalar.copy(Qn[:], pq[:])
                        else:
                            Qn = None
                        nc.tensor.matmul(pR[:], Pn[:], Rm[:], start=False,
                                         stop=True,
                                         skip_group_check=True)
                        Rn = recp.tile([128, 128], bf16,
                                       name=f"Rm{head}{ch}{lvl + 1}", tag="Rm")
                        nc.scalar.copy(Rn[:], pR[:])
                        Rm, Pm, Qm = Rn, Pn, Qn

                    # RHS' = V - K S    (psum = K@S, then V - psum on DVE)
                    pks = psw.tile([128, 256], fp32, name=f"pks{head}{ch}",
                                   tag="w")
                    for i in range(2):
                        nc.tensor.matmul(pks[:], KTt[i],
                                         s_sb[:, i * 256:(i + 1) * 256],
                                         start=(i == 0), stop=(i == 1))
                    rhs_sb = recp.tile([128, 256], bf16, name=f"rhs{head}{ch}",
                                       tag="rhs")
                    nc.vector.tensor_sub(rhs_sb[:], vcd[:], pks[:])

                    # U' via block forward substitution (4 blocks of 32)
                    u_sb = recp.tile([128, 256], bf16, name=f"u{head}{ch}",
                                     tag="u")
                    y_sb = recp.tile([128, 256], bf16, name=f"y{head}{ch}",
                                     tag="y")
                    nc.vector.memset(u_sb[:], 0.0)
                    px = psw.tile([128, 256], fp32, name=f"px{head}{ch}",
                                  tag="w")
                    py = psw.tile([128, 256], fp32, name=f"py{head}{ch}",
                                  tag="w")
                    nc.tensor.matmul(px[0:32, :], Rm[0:32, 0:32],
                                     rhs_sb[0:32, :], start=True, stop=True,
                                     tile_position=(0, 0))
                    nc.vector.tensor_copy(u_sb[0:32, :], px[0:32, :])
                    for i in range(1, 4):
                        p0 = 32 * i
                        nc.tensor.matmul(py[p0:p0 + 32, :],
                                         FnT[:, p0:p0 + 32], u_sb[:],
                                         start=True, stop=True,
                                         tile_position=(0, p0))
                        nc.vector.tensor_add(y_sb[p0:p0 + 32, :],
                                             rhs_sb[p0:p0 + 32, :],
                                             py[p0:p0 + 32, :])
                        nc.tensor.matmul(px[p0:p0 + 32, :],
                                         Rm[p0:p0 + 32, p0:p0 + 32],
                                         y_sb[p0:p0 + 32, :],
                                         start=True, stop=True,
                                         tile_position=(p0, p0))
                        nc.vector.tensor_copy(u_sb[p0:p0 + 32, :],
                                              px[p0:p0 + 32, :])

                    # attn P = triu_incl(K Q^T)
                    pkq = psw.tile([128, 128], fp32, name=f"pkq{head}{ch}",
                                   tag="w")
                    for i in range(2):
                        nc.tensor.matmul(pkq[:], KTt[i], QT[i], start=(i == 0),
                                         stop=(i == 1))
                    Pat = recp.tile([128, 128], bf16, name=f"Pat{head}{ch}",
                                    tag="Pat")
                    nc.vector.tensor_mul(Pat[:], pkq[:], m_triuI)

                    # O = Q S + P^T U'
                    po = psw.tile([128, 256], fp32, name=f"po{head}{ch}",
                                  tag="w")
                    for i in range(2):
                        nc.tensor.matmul(po[:], QT[i],
                                         s_sb[:, i * 256:(i + 1) * 256],
                                         start=(i == 0), stop=False)
                    nc.tensor.matmul(po[:], Pat[:], u_sb[:], start=False,
                                     stop=True)

                    # S += K^T U'   (accumulate in persistent psum)
                    for i in range(2):
                        nc.tensor.matmul(s_ps[:, i * 256:(i + 1) * 256],
                                         kcd[:, i * 128:(i + 1) * 128], u_sb[:],
                                         start=(ch == 0 and i == 0), stop=True,
                                         skip_group_check=True)
                    s_nb = recs.tile([128, 512], bf16, name=f"ssb{head}{ch}",
                                     tag="ssb", bufs=2)
                    nc.vector.tensor_copy(s_nb[:], s_ps[:])
                    s_sb = s_nb

                    # RMSNorm rows of O, then transpose out to oT
                    osq = recp.tile([128, 256], bf16, name=f"osq{head}{ch}",
                                    tag="osq")
                    ossq = recp.tile([128, 1], fp32, name=f"ossq{head}{ch}",
                                     tag="ossq")
                    nc.scalar.activation(osq[:], po[:], AF.Square,
                                         accum_out=ossq[:])
                    orsq = recp.tile([128, 1], fp32, name=f"orsq{head}{ch}",
                                     tag="orsq")
                    nc.scalar.activation(orsq[:], ossq[:], AF.Sqrt,
                                         bias=biases[:, 1:2], scale=1.0 / DH)
                    nc.vector.reciprocal(orsq[:], orsq[:])
                    onrm = recp.tile([128, 256], bf16, name=f"onrm{head}{ch}",
                                     tag="onrm")
                    nc.vector.tensor_scalar_mul(onrm[:], po[:], orsq[:])
                    for i in range(2):
                        pto = psw.tile([128, 128], bf16,
                                       name=f"pto{head}{ch}{i}", tag="w")
                        nc.tensor.transpose(pto[:], onrm[:, i * 128:(i + 1) * 128],
                                            ident)
                        nc.scalar.copy(oT[ct0 + i][:, t0:t0 + C], pto[:])

        # ================= phase C: output projection =====================
        if DEBUG_SKIP_WO:
            return
        with tc.tile_pool(name="wop", bufs=CT) as wop, \
             tc.tile_pool(name="ofp", bufs=3) as ofp:
            wo_s = []
            for ct in range(CT):
                t_ = wop.tile([128, D], bf16, name=f"wo{ct}", tag="wo")
                nc.sync.dma_start(t_[:], wo_t[ct])
                wo_s.append(t_)
            for tt in range(T // 128):
                for half in range(2):
                    pf = psw.tile([128, 512], fp32, name=f"pf{tt}{half}",
                                  tag="w")
                    for ct in range(CT):
                        nc.tensor.matmul(
                            pf[:], oT[ct][:, tt * 128:(tt + 1) * 128],
                            wo_s[ct][:, half * 512:(half + 1) * 512],
                            start=(ct == 0), stop=(ct == CT - 1))
                    of = ofp.tile([128, 512], fp32, name=f"of{tt}{half}",
                                  tag="of")
                    nc.scalar.copy(of[:], pf[:])
                    nc.sync.dma_start(
                        out_t[tt][:, half * 512:(half + 1) * 512], of[:])


LP_NP = np.float16  # host-side 16-bit dtype matching the device dtype


def _make_consts():
    ii = np.arange(128)
    blk = ii[:, None] // 32 == ii[None, :] // 32
    ident = np.eye(128, dtype=np.float32)
    bdl = ((ii[:, None] > ii[None, :]) & blk).astype(np.float32)
    bdu = ((ii[:, None] < ii[None, :]) & blk).astype(np.float32)
    fneg = -((ii[:, None] < ii[None, :]) & ~blk).astype(np.float32)
    triuI = (ii[:, None] <= ii[None, :]).astype(np.float32)
    return np.concatenate([ident, bdl, bdu, -bdl, fneg, triuI],
                          axis=1).astype(LP_NP)


def _get_compiled():
    key = ("nc", SILU_NATIVE)
    if key not in _CACHE:
        _CACHE[key] = _build_bass()
    return _CACHE[key]


def kernel(hidden_states, Wq, Wk, Wv, conv_wq, conv_wk, conv_wv, onorm_w, Wo):
    from concourse.bass_utils import run_bass_kernel_spmd

    hidden_states = np.asarray(hidden_states, np.float32)
    Wq = np.asarray(Wq, np.float32)
    Wk = np.asarray(Wk, np.float32)
    Wv = np.asarray(Wv, np.float32)
    Wo = np.asarray(Wo, np.float32)
    conv_wq = np.asarray(conv_wq, np.float32)
    conv_wk = np.asarray(conv_wk, np.float32)
    conv_wv = np.asarray(conv_wv, np.float32)
    onorm_w = np.asarray(onorm_w, np.float32)

    bf = LP_NP
    consts = _make_consts()
    Wo_eff = (Wo * np.tile(onorm_w, H)[:, None]).astype(bf)  # fold RMS weight

    in_maps = []
    for core in range(NCORES):
        b, g = divmod(core, 2)
        cols = slice(CG * g, CG * (g + 1))
        in_maps.append({
            "xT": np.ascontiguousarray(hidden_states[b].T).astype(bf),
            "wq": np.ascontiguousarray(Wq[:, cols]).astype(bf),
            "wk": np.ascontiguousarray(Wk[:, cols]).astype(bf),
            "wv": np.ascontiguousarray(Wv[:, cols]).astype(bf),
            "wo": np.ascontiguousarray(Wo_eff[cols, :]),
            "cw": np.ascontiguousarray(np.concatenate(
                [conv_wq[cols], conv_wk[cols], conv_wv[cols]], axis=1)),
            "consts": consts,
        })

    nc = _get_compiled()
    res = run_bass_kernel_spmd(nc, in_maps, core_ids=list(range(NCORES)),
                               **_CACHE.get("run_kwargs", {}))
    _CACHE["last_results"] = res
    out = np.zeros((B, T, D), np.float32)
    for core in range(NCORES):
        out[core // 2] += res.results[core]["out"]
    return out



# revision 4
# speedup vs baseline: 1.1104x; 1.1104x over previous
"""DeltaNet forward kernel for 8 Trainium2 NeuronCores.

Problem (hardcoded from the task spec): hidden_states [B=4, T=2048, D=1024],
H=4 heads, Dh=256, causal depthwise conv K=4 + silu on q/k/v projections,
q/k l2-normalized per head (q scaled Dh^-0.5), delta-rule recurrence over T,
per-head RMSNorm, merge heads, out = o @ Wo.

Sharding: data-parallel over (batch, head-group): core c -> batch c//2,
head group c%2 (projection columns [512*(c%2), 512*(c%2)+512)). Each core
computes a partial product against its 512 rows of Wo; the host sums the two
partials per batch (the unshard step for the row-parallel output matmul).

Device algorithm: chunked WY form of the delta rule, chunk C=128, the two
heads of the group interleaved chunk-by-chunk so their independent dependency
chains fill each other's stalls. Per chunk (per head): KK = K K^T;
R ~= (I+B)^{-1} for the 64-block-diagonal strict-upper B via masked Neumann
doubling (I-B)(I+B^2)(I+B^4)(I+B^8)(I+B^16); U' via one 64-block forward-
substitution step; O = Q S + triu(K Q^T)^T U'; S += K^T U'. S accumulates in
PSUM f32; matmul operands are 16-bit. The output projection (o @ Wo) runs
inside the chunk loop so it overlaps the recurrence.
"""

import numpy as np

B, T, D = 4, 2048, 1024
H = 4
DH = D // H          # 256
CONV_K = 4
EPS = 1e-5
NCORES = 8
CG = 512             # columns per core (2 heads)
C = 128              # recurrence chunk length
NCHUNK = T // C      # 16
PAD = 4              # front zero padding on time axis for the causal conv
TOKB = 512           # token block (matmul moving size)
KT = D // 128        # 8 contraction tiles
CT = CG // 128       # 4 column tiles per core
NB = T // TOKB       # 4 token blocks

_CACHE = {}
SILU_NATIVE = True  # CoreSim lacks Silu; set False for simulation runs
DEBUG_SKIP_WO = False  # debug: skip output projection phase


def _build_bass():
    import concourse.bass as bass  # noqa: F401
    import concourse.bacc as bacc
    import concourse.mybir as mybir
    import concourse.tile as tile

    dt = mybir.dt
    nc = bacc.Bacc("TRN2", target_bir_lowering=False, debug=False)

    xT = nc.dram_tensor("xT", [D, T], dt.float16, kind="ExternalInput")
    wq = nc.dram_tensor("wq", [D, CG], dt.float16, kind="ExternalInput")
    wk = nc.dram_tensor("wk", [D, CG], dt.float16, kind="ExternalInput")
    wv = nc.dram_tensor("wv", [D, CG], dt.float16, kind="ExternalInput")
    wo = nc.dram_tensor("wo", [CG, D], dt.float16, kind="ExternalInput")
    cw = nc.dram_tensor("cw", [CG, 3 * CONV_K], dt.float32, kind="ExternalInput")
    consts = nc.dram_tensor("consts", [128, 6 * 128], dt.float16,
                            kind="ExternalInput")
    out = nc.dram_tensor("out", [T, D], dt.float32, kind="ExternalOutput")

    with tile.TileContext(nc) as tc:
        _body(nc, tc, mybir, xT, wq, wk, wv, wo, cw, consts, out)

    nc.compile()
    return nc


def _body(nc, tc, mybir, xT, wq, wk, wv, wo, cw, consts, out):
    dt = mybir.dt
    AF = mybir.ActivationFunctionType
    ALU = mybir.AluOpType
    fp32 = dt.float32
    bf16 = dt.float16  # 16-bit working dtype (fp16: 11-bit mantissa)
    NT = T + PAD

    xT_t = xT.ap().rearrange("(n p) t -> n p t", p=128)       # [8,128,T]
    w_t = {"q": wq.ap().rearrange("(n p) c -> n p c", p=128),
           "k": wk.ap().rearrange("(n p) c -> n p c", p=128),
           "v": wv.ap().rearrange("(n p) c -> n p c", p=128)}
    wo_t = wo.ap().rearrange("(n p) c -> n p c", p=128)       # [4,128,D]
    cw_t = cw.ap().rearrange("(n p) c -> n p c", p=128)       # [4,128,12]
    out_t = out.ap().rearrange("(n p) c -> n p c", p=128)     # [16,128,D]

    # ---------- persistent pool (lives for the whole kernel) ----------
    with tc.tile_pool(name="persist", bufs=1) as persist, \
         tc.tile_pool(name="qkvp", bufs=3 * CT) as qkvp, \
         tc.tile_pool(name="wop", bufs=CT) as wop, \
         tc.tile_pool(name="psw", bufs=6, space="PSUM") as psw, \
         tc.tile_pool(name="pss", bufs=2, space="PSUM") as pss:

        cons = persist.tile([128, 6 * 128], bf16, name="cons", tag="cons")
        nc.sync.dma_start(cons[:], consts.ap())
        ident = cons[:, 0:128]          # identity
        m_bdl = cons[:, 128:256]        # block-diag(64) strict lower, +1
        m_bdu = cons[:, 256:384]        # block-diag(64) strict upper, +1
        m_fneg = cons[:, 384:512]       # strict upper outside blocks, -1
        m_triuI = cons[:, 512:640]      # i<=j, +1
        ones128 = cons[:, 640:768]      # all ones

        biases = persist.tile([128, 3], dt.float32, name="biases", tag="biases")
        nc.vector.memset(biases[:, 0:1], 1e-6)
        nc.vector.memset(biases[:, 1:2], EPS)
        nc.vector.memset(biases[:, 2:3], 1e-6 * DH)

        cwt = []
        for ct in range(CT):
            t_ = persist.tile([128, 3 * CONV_K], fp32, name=f"cw{ct}",
                              tag=f"cw{ct}")
            nc.sync.dma_start(t_[:], cw_t[ct])
            cwt.append(t_)

        wo_s = []
        for ct in range(CT):
            t_ = wop.tile([128, D], bf16, name=f"wo{ct}", tag="wo")
            nc.sync.dma_start(t_[:], wo_t[ct])
            wo_s.append(t_)

        qh, kh, vh = [], [], []
        for lst, nm in ((qh, "q"), (kh, "k"), (vh, "v")):
            for ct in range(CT):
                lst.append(qkvp.tile([128, T], bf16, name=f"{nm}hat{ct}",
                                     tag="qkv"))

        # ================= phase A: projections + conv + silu + l2norm ====
        with tc.tile_pool(name="xp", bufs=KT) as xp, \
             tc.tile_pool(name="wp", bufs=3 * KT) as wp, \
             tc.tile_pool(name="rawp", bufs=4) as rawp, \
             tc.tile_pool(name="sqp", bufs=4) as sqp, \
             tc.tile_pool(name="bcp", bufs=3) as bcp:

            xt = []
            for kt in range(KT):
                t_ = xp.tile([128, T], bf16, name=f"xt{kt}", tag="xt")
                nc.sync.dma_start(t_[:], xT_t[kt])
                xt.append(t_)
            ws = {}
            for nm in ("q", "k", "v"):
                ws[nm] = []
                for kt in range(KT):
                    t_ = wp.tile([128, CG], bf16, name=f"w{nm}{kt}", tag="w")
                    nc.sync.dma_start(t_[:], w_t[nm][kt])
                    ws[nm].append(t_)

            for ti, (nm, dest) in enumerate((("q", qh), ("k", kh), ("v", vh))):
                sq_tiles = []
                for ct in range(CT):
                    rawt = rawp.tile([128, NT], bf16, name=f"raw{nm}{ct}",
                                     tag="raw")
                    nc.vector.memset(rawt[:, 0:PAD], 0.0)
                    dst = dest[ct]
                    for nb in range(NB):
                        pt = psw.tile([128, TOKB], fp32, name=f"pp{nm}{ct}{nb}",
                                      tag="w")
                        for kt in range(KT):
                            nc.tensor.matmul(
                                pt[:], ws[nm][kt][:, ct * 128:(ct + 1) * 128],
                                xt[kt][:, nb * TOKB:(nb + 1) * TOKB],
                                start=(kt == 0), stop=(kt == KT - 1))
                        nc.scalar.copy(
                            rawt[:, PAD + nb * TOKB:PAD + (nb + 1) * TOKB],
                            pt[:])
                    # causal depthwise conv along t
                    w0 = cwt[ct][:, ti * CONV_K:ti * CONV_K + 1]
                    nc.vector.tensor_scalar_mul(dst[:], rawt[:, 1:1 + T], w0)
                    for i in range(1, CONV_K):
                        wi = cwt[ct][:, ti * CONV_K + i:ti * CONV_K + i + 1]
                        nc.vector.scalar_tensor_tensor(
                            dst[:], rawt[:, 1 + i:1 + i + T], wi, dst[:],
                            ALU.mult, ALU.add)
                    if SILU_NATIVE:
                        nc.scalar.activation(dst[:], dst[:], AF.Silu)
                    else:
                        sg = rawp.tile([128, T], bf16, name=f"sg{nm}{ct}",
                                       tag="raw")
                        nc.scalar.activation(sg[:], dst[:], AF.Sigmoid)
                        nc.vector.tensor_mul(dst[:], dst[:], sg[:])
                    if ti < 2:
                        sqt = sqp.tile([128, T], bf16, name=f"sq{nm}{ct}",
                                       tag="sq")
                        nc.scalar.activation(sqt[:], dst[:], AF.Square)
                        sq_tiles.append(sqt)
                if ti < 2:
                    # per-head l2norm: ones-matrix matmul gives the per-token
                    # sum of squares broadcast to all 128 partitions at once;
                    # rsq = scale/sqrt(ss + 1e-6) applied per token block.
                    for head in range(2):
                        for nb in range(NB):
                            bc = psw.tile([128, TOKB], fp32,
                                          name=f"bc{nm}{head}{nb}", tag="w")
                            for cth in range(2):
                                nc.tensor.matmul(
                                    bc[:], ones128,
                                    sq_tiles[head * 2 + cth][
                                        :, nb * TOKB:(nb + 1) * TOKB],
                                    start=(cth == 0), stop=(cth == 1))
                            bcf = bcp.tile([128, TOKB], fp32,
                                           name=f"bcf{nm}{head}{nb}",
                                           tag="bcf")
                            if ti == 0:
                                # fold Dh^-0.5: 1/(16 sqrt(ss+eps)) =
                                # 1/sqrt(256 ss + 256 eps)
                                nc.scalar.activation(bcf[:], bc[:], AF.Sqrt,
                                                     bias=biases[:, 2:3],
                                                     scale=float(DH))
                            else:
                                nc.scalar.activation(bcf[:], bc[:], AF.Sqrt,
                                                     bias=biases[:, 0:1])
                            nc.vector.reciprocal(bcf[:], bcf[:])
                            bcb = bcp.tile([128, TOKB], bf16,
                                           name=f"bcb{nm}{head}{nb}",
                                           tag="bcb")
                            nc.scalar.copy(bcb[:], bcf[:])
                            sl = slice(nb * TOKB, (nb + 1) * TOKB)
                            for cth in range(2):
                                ct = head * 2 + cth
                                nc.vector.tensor_mul(dest[ct][:, sl],
                                                     dest[ct][:, sl], bcb[:])

        # ====== phase B + C: delta-rule recurrence, heads interleaved =====
        with tc.tile_pool(name="recp", bufs=1) as recp, \
             tc.tile_pool(name="otp", bufs=4) as otp, \
             tc.tile_pool(name="ofp", bufs=3) as ofp:
            s_ps, s_sb = [], []
            for head in range(2):
                s_ps.append(pss.tile([128, 512], fp32, name=f"sps{head}",
                                     tag="sps"))
                t_ = recp.tile([128, 512], bf16, name=f"ssb{head}", tag="ssb",
                               bufs=4)
                nc.vector.memset(t_[:], 0.0)
                s_sb.append(t_)
            oTc = [None, None]

            for ch in range(NCHUNK):
                t0 = ch * C
                for head in range(2):
                    ct0 = head * 2
                    QT = [qh[ct0][:, t0:t0 + C], qh[ct0 + 1][:, t0:t0 + C]]
                    KTt = [kh[ct0][:, t0:t0 + C], kh[ct0 + 1][:, t0:t0 + C]]
                    VT = [vh[ct0][:, t0:t0 + C], vh[ct0 + 1][:, t0:t0 + C]]

                    # K, V in [C, Dh] layout via PE transpose (bf16 psum)
                    ptkv = psw.tile([128, 512], bf16, name=f"ptkv{head}{ch}",
                                    tag="w")
                    for i in range(2):
                        nc.tensor.transpose(ptkv[:, i * 128:(i + 1) * 128],
                                            KTt[i], ident)
                        nc.tensor.transpose(
                            ptkv[:, 256 + i * 128:256 + (i + 1) * 128],
                            VT[i], ident)
                    kvcd = recp.tile([128, 512], bf16, name=f"kvcd{head}{ch}",
                                     tag="kvcd", bufs=3)
                    nc.vector.tensor_copy(kvcd[:], ptkv[:])

                    # KK^T and its masked pieces (SBUF bf16, 2x DVE mode)
                    pkk = psw.tile([128, 128], fp32, name=f"pkk{head}{ch}",
                                   tag="w")
                    for i in range(2):
                        nc.tensor.matmul(pkk[:], KTt[i], KTt[i], start=(i == 0),
                                         stop=(i == 1))
                    pkkS = recp.tile([128, 128], bf16, name=f"pkkS{head}{ch}",
                                     tag="pkkS", bufs=3)
                    nc.scalar.copy(pkkS[:], pkk[:])
                    Nl = recp.tile([128, 128], bf16, name=f"Nl{head}{ch}",
                                   tag="Nl", bufs=3)
                    Nu = recp.tile([128, 128], bf16, name=f"Nu{head}{ch}",
                                   tag="Nu", bufs=3)
                    FnT = recp.tile([128, 128], bf16, name=f"FnT{head}{ch}",
                                    tag="FnT", bufs=3)
                    R0 = recp.tile([128, 128], bf16, name=f"R0{head}{ch}",
                                   tag="R0", bufs=3)
                    nc.vector.tensor_mul(Nl[:], pkkS[:], m_bdl)
                    nc.vector.tensor_mul(Nu[:], pkkS[:], m_bdu)
                    nc.vector.tensor_mul(FnT[:], pkkS[:], m_fneg)
                    nc.vector.tensor_sub(R0[:], ident, Nu[:])

                    # R ~= (I+B)^{-1} on 64-blocks: Neumann doubling to B^16
                    pR = psw.tile([128, 128], fp32, name=f"pR{head}{ch}",
                                  tag="w")
                    nc.tensor.matmul(pR[:], ident, R0[:], start=True,
                                     stop=True)
                    Rm, Pm, Qm = R0, Nl, Nu
                    for lvl in range(4):
                        ppq = psw.tile([128, 256], fp32,
                                       name=f"ppq{head}{ch}{lvl}", tag="w")
                        nc.tensor.matmul(ppq[:, 0:128], Qm[:], Pm[:],
                                         start=True, stop=True)
                        Pn = recp.tile([128, 128], bf16,
                                       name=f"Pn{head}{ch}{lvl}", tag="Pn",
                                       bufs=3)
                        nc.vector.tensor_copy(Pn[:], ppq[:, 0:128])
                        if lvl < 3:
                            nc.tensor.matmul(ppq[:, 128:256], Pm[:], Qm[:],
                                             start=True, stop=True)
                            Qn = recp.tile([128, 128], bf16,
                                           name=f"Qn{head}{ch}{lvl}", tag="Qn",
                                           bufs=3)
                            nc.scalar.copy(Qn[:], ppq[:, 128:256])
                        else:
                            Qn = None
                        nc.tensor.matmul(pR[:], Pn[:], Rm[:], start=False,
                                         stop=True, skip_group_check=True)
                        Rn = recp.tile([128, 128], bf16,
                                       name=f"Rm{head}{ch}{lvl}", tag="Rm",
                                       bufs=3)
                        nc.scalar.copy(Rn[:], pR[:])
                        Rm, Pm, Qm = Rn, Pn, Qn

                    # RHS' = V - K S    (psum = K@S, then V - psum on DVE)
                    pks = psw.tile([128, 256], fp32, name=f"pks{head}{ch}",
                                   tag="w")
                    for i in range(2):
                        nc.tensor.matmul(pks[:], KTt[i],
                                         s_sb[head][:, i * 256:(i + 1) * 256],
                                         start=(i == 0), stop=(i == 1))
                    rhs_sb = recp.tile([128, 256], bf16, name=f"rhs{head}{ch}",
                                       tag="rhs", bufs=3)
                    nc.vector.tensor_sub(rhs_sb[:], kvcd[:, 256:512], pks[:])

                    # U' via one 64-block forward-substitution step
                    u_sb = recp.tile([128, 256], bf16, name=f"u{head}{ch}",
                                     tag="u", bufs=3)
                    y_sb = recp.tile([128, 256], bf16, name=f"y{head}{ch}",
                                     tag="y", bufs=3)
                    px = psw.tile([128, 256], fp32, name=f"px{head}{ch}",
                                  tag="w")
                    py = psw.tile([128, 256], fp32, name=f"py{head}{ch}",
                                  tag="w")
                    nc.tensor.matmul(px[0:64, :], Rm[0:64, 0:64],
                                     rhs_sb[0:64, :], start=True, stop=True,
                                     tile_position=(0, 0))
                    nc.vector.tensor_copy(u_sb[0:64, :], px[0:64, :])
                    nc.tensor.matmul(py[64:128, :], FnT[0:64, 64:128],
                                     u_sb[0:64, :], start=True, stop=True,
                                     tile_position=(0, 64))
                    nc.vector.tensor_add(y_sb[64:128, :], rhs_sb[64:128, :],
                                         py[64:128, :])
                    nc.tensor.matmul(px[64:128, :], Rm[64:128, 64:128],
                                     y_sb[64:128, :], start=True, stop=True,
                                     tile_position=(64, 64))
                    nc.vector.tensor_copy(u_sb[64:128, :], px[64:128, :])

                    # attn P = triu_incl(K Q^T)
                    pkq = psw.tile([128, 128], fp32, name=f"pkq{head}{ch}",
                                   tag="w")
                    for i in range(2):
                        nc.tensor.matmul(pkq[:], KTt[i], QT[i], start=(i == 0),
                                         stop=(i == 1))
                    pkqS = recp.tile([128, 128], bf16, name=f"pkqS{head}{ch}",
                                     tag="pkqS", bufs=3)
                    nc.scalar.copy(pkqS[:], pkq[:])
                    Pat = recp.tile([128, 128], bf16, name=f"Pat{head}{ch}",
                                    tag="Pat", bufs=3)
                    nc.vector.tensor_mul(Pat[:], pkqS[:], m_triuI)

                    # O = Q S + P^T U'
                    po = psw.tile([128, 256], fp32, name=f"po{head}{ch}",
                                  tag="w")
                    for i in range(2):
                        nc.tensor.matmul(po[:], QT[i],
                                         s_sb[head][:, i * 256:(i + 1) * 256],
                                         start=(i == 0), stop=False)
                    nc.tensor.matmul(po[:], Pat[:], u_sb[:], start=False,
                                     stop=True)

                    # S += K^T U'   (accumulate in persistent psum)
                    for i in range(2):
                        nc.tensor.matmul(s_ps[head][:, i * 256:(i + 1) * 256],
                                         kvcd[:, i * 128:(i + 1) * 128],
                                         u_sb[:],
                                         start=(ch == 0 and i == 0), stop=True,
                                         skip_group_check=True)
                    s_nb = recp.tile([128, 512], bf16, name=f"ssb{head}{ch}",
                                     tag="ssb", bufs=4)
                    nc.scalar.copy(s_nb[:], s_ps[head][:])
                    s_sb[head] = s_nb

                    # RMSNorm rows of O, then transpose out
                    osq = recp.tile([128, 256], bf16, name=f"osq{head}{ch}",
                                    tag="osq", bufs=3)
                    ossq = recp.tile([128, 1], fp32, name=f"ossq{head}{ch}",
                                     tag="ossq", bufs=3)
                    nc.scalar.activation(osq[:], po[:], AF.Square,
                                         accum_out=ossq[:])
                    orsq = recp.tile([128, 1], fp32, name=f"orsq{head}{ch}",
                                     tag="orsq", bufs=3)
                    nc.scalar.activation(orsq[:], ossq[:], AF.Sqrt,
                                         bias=biases[:, 1:2], scale=1.0 / DH)
                    nc.vector.reciprocal(orsq[:], orsq[:])
                    onrm = recp.tile([128, 256], bf16, name=f"onrm{head}{ch}",
                                     tag="onrm", bufs=3)
                    nc.vector.tensor_scalar_mul(onrm[:], po[:], orsq[:])
                    pto = psw.tile([128, 256], bf16, name=f"pto{head}{ch}",
                                   tag="w")
                    for i in range(2):
                        nc.tensor.transpose(pto[:, i * 128:(i + 1) * 128],
                                            onrm[:, i * 128:(i + 1) * 128],
                                            ident)
                    oTt = otp.tile([128, 256], bf16, name=f"oT{head}{ch}",
                                   tag="oT")
                    nc.vector.tensor_copy(oTt[:], pto[:])
                    oTc[head] = oTt

                # ---- output projection for this chunk (both heads) ----
                if DEBUG_SKIP_WO:
                    continue
                for half in range(2):
                    pf = psw.tile([128, 512], fp32, name=f"pf{ch}{half}",
                                  tag="w")
                    k = 0
                    for hd in range(2):
                        for i in range(2):
                            nc.tensor.matmul(
                                pf[:], oTc[hd][:, i * 128:(i + 1) * 128],
                                wo_s[hd * 2 + i][:, half * 512:(half + 1) * 512],
                                start=(k == 0), stop=(k == 3))
                            k += 1
                    of = ofp.tile([128, 512], fp32, name=f"of{ch}{half}",
                                  tag="of")
                    nc.scalar.copy(of[:], pf[:])
                    nc.sync.dma_start(
                        out_t[ch][:, half * 512:(half + 1) * 512], of[:])


LP_NP = np.float16  # host-side 16-bit dtype matching the device dtype


def _make_consts():
    ii = np.arange(128)
    blk = ii[:, None] // 64 == ii[None, :] // 64
    ident = np.eye(128, dtype=np.float32)
    bdl = ((ii[:, None] > ii[None, :]) & blk).astype(np.float32)
    bdu = ((ii[:, None] < ii[None, :]) & blk).astype(np.float32)
    fneg = -((ii[:, None] < ii[None, :]) & ~blk).astype(np.float32)
    triuI = (ii[:, None] <= ii[None, :]).astype(np.float32)
    ones = np.ones((128, 128), np.float32)
    return np.concatenate([ident, bdl, bdu, fneg, triuI, ones],
                          axis=1).astype(LP_NP)


def _get_compiled():
    key = ("nc", SILU_NATIVE)
    if key not in _CACHE:
        _CACHE[key] = _build_bass()
    return _CACHE[key]


def kernel(hidden_states, Wq, Wk, Wv, conv_wq, conv_wk, conv_wv, onorm_w, Wo):
    from concourse.bass_utils import run_bass_kernel_spmd

    hidden_states = np.asarray(hidden_states, np.float32)
    Wq = np.asarray(Wq, np.float32)
    Wk = np.asarray(Wk, np.float32)
    Wv = np.asarray(Wv, np.float32)
    Wo = np.asarray(Wo, np.float32)
    conv_wq = np.asarray(conv_wq, np.float32)
    conv_wk = np.asarray(conv_wk, np.float32)
    conv_wv = np.asarray(conv_wv, np.float32)
    onorm_w = np.asarray(onorm_w, np.float32)

    bf = LP_NP
    consts = _make_consts()
    Wo_eff = (Wo * np.tile(onorm_w, H)[:, None]).astype(bf)  # fold RMS weight

    in_maps = []
    for core in range(NCORES):
        b, g = divmod(core, 2)
        cols = slice(CG * g, CG * (g + 1))
        in_maps.append({
            "xT": np.ascontiguousarray(hidden_states[b].T).astype(bf),
            "wq": np.ascontiguousarray(Wq[:, cols]).astype(bf),
            "wk": np.ascontiguousarray(Wk[:, cols]).astype(bf),
            "wv": np.ascontiguousarray(Wv[:, cols]).astype(bf),
            "wo": np.ascontiguousarray(Wo_eff[cols, :]),
            "cw": np.ascontiguousarray(np.concatenate(
                [conv_wq[cols], conv_wk[cols], conv_wv[cols]], axis=1)),
            "consts": consts,
        })

    nc = _get_compiled()
    res = run_bass_kernel_spmd(nc, in_maps, core_ids=list(range(NCORES)),
                               **_CACHE.get("run_kwargs", {}))
    _CACHE["last_results"] = res
    out = np.zeros((B, T, D), np.float32)
    for core in range(NCORES):
        out[core // 2] += res.results[core]["out"]
    return out


# revision 7
# speedup vs baseline: 1.5862x; 1.4286x over previous
"""DeltaNet forward kernel for 8 Trainium2 NeuronCores.

Problem (hardcoded from the task spec): hidden_states [B=4, T=2048, D=1024],
H=4 heads, Dh=256, causal depthwise conv K=4 + silu on q/k/v projections,
q/k l2-normalized per head (q scaled Dh^-0.5), delta-rule recurrence over T,
per-head RMSNorm, merge heads, out = o @ Wo.

Sharding: data-parallel over (batch, head-group): core c -> batch c//2,
head group c%2 (projection columns [512*(c%2), 512*(c%2)+512)). Each core
computes a partial product against its 512 rows of Wo; the host sums the two
partials per batch (the unshard step for the row-parallel output matmul).

Device algorithm: chunked WY form of the delta rule, chunk C=128, the two
heads of the group interleaved chunk-by-chunk so their independent dependency
chains fill each other's stalls. Per chunk (per head): KK = K K^T;
R ~= (I+B)^{-1} for the 64-block-diagonal strict-upper B via masked Neumann
doubling (I-B)(I+B^2)(I+B^4)(I+B^8)(I+B^16); U' via one 64-block forward-
substitution step; O = Q S + triu(K Q^T)^T U'; S += K^T U'. S accumulates in
PSUM f32; matmul operands are 16-bit. The output projection (o @ Wo) runs
inside the chunk loop so it overlaps the recurrence.
"""

import numpy as np

B, T, D = 4, 2048, 1024
H = 4
DH = D // H          # 256
CONV_K = 4
EPS = 1e-5
NCORES = 8
CG = 512             # columns per core (2 heads)
C = 128              # recurrence chunk length
NCHUNK = T // C      # 16
PAD = 4              # front zero padding on time axis for the causal conv
TOKB = 512           # token block (matmul moving size)
KT = D // 128        # 8 contraction tiles
CT = CG // 128       # 4 column tiles per core
NB = T // TOKB       # 4 token blocks

_CACHE = {}
SILU_NATIVE = True  # CoreSim lacks Silu; set False for simulation runs
DEBUG_SKIP_WO = False  # debug: skip output projection phase


def _build_bass():
    import concourse.bass as bass  # noqa: F401
    import concourse.bacc as bacc
    import concourse.mybir as mybir
    import concourse.tile as tile

    dt = mybir.dt
    nc = bacc.Bacc("TRN2", target_bir_lowering=False, debug=False)

    xT = nc.dram_tensor("xT", [D, T], dt.float16, kind="ExternalInput")
    wq = nc.dram_tensor("wq", [D, CG], dt.float16, kind="ExternalInput")
    wk = nc.dram_tensor("wk", [D, CG], dt.float16, kind="ExternalInput")
    wv = nc.dram_tensor("wv", [D, CG], dt.float16, kind="ExternalInput")
    wo = nc.dram_tensor("wo", [CG, D], dt.float16, kind="ExternalInput")
    cw = nc.dram_tensor("cw", [CG, 3 * CONV_K], dt.float32, kind="ExternalInput")
    consts = nc.dram_tensor("consts", [128, 6 * 128], dt.float16,
                            kind="ExternalInput")
    out = nc.dram_tensor("out", [T, D], dt.float32, kind="ExternalOutput")

    with tile.TileContext(nc) as tc:
        _body(nc, tc, mybir, xT, wq, wk, wv, wo, cw, consts, out)

    nc.compile()
    return nc


def _body(nc, tc, mybir, xT, wq, wk, wv, wo, cw, consts, out):
    dt = mybir.dt
    AF = mybir.ActivationFunctionType
    ALU = mybir.AluOpType
    fp32 = dt.float32
    bf16 = dt.float16  # 16-bit working dtype (fp16: 11-bit mantissa)
    NT = T + PAD

    xT_t = xT.ap().rearrange("(n p) t -> n p t", p=128)       # [8,128,T]
    w_t = {"q": wq.ap().rearrange("(n p) c -> n p c", p=128),
           "k": wk.ap().rearrange("(n p) c -> n p c", p=128),
           "v": wv.ap().rearrange("(n p) c -> n p c", p=128)}
    wo_t = wo.ap().rearrange("(n p) c -> n p c", p=128)       # [4,128,D]
    cw_t = cw.ap().rearrange("(n p) c -> n p c", p=128)       # [4,128,12]
    out_t = out.ap().rearrange("(n p) c -> n p c", p=128)     # [16,128,D]

    # ---------- persistent pool (lives for the whole kernel) ----------
    with tc.tile_pool(name="persist", bufs=1) as persist, \
         tc.tile_pool(name="qkvp", bufs=3 * CT) as qkvp, \
         tc.tile_pool(name="wop", bufs=CT) as wop, \
         tc.tile_pool(name="psw", bufs=6, space="PSUM") as psw, \
         tc.tile_pool(name="pss", bufs=2, space="PSUM") as pss:

        cons = persist.tile([128, 6 * 128], bf16, name="cons", tag="cons")
        nc.sync.dma_start(cons[:], consts.ap())
        ident = cons[:, 0:128]          # identity
        m_bdl = cons[:, 128:256]        # strict lower, +1
        m_bdu = cons[:, 256:384]        # strict upper, +1
        m_triuI = cons[:, 512:640]      # i<=j, +1
        ones128 = cons[:, 640:768]      # all ones

        biases = persist.tile([128, 3], dt.float32, name="biases", tag="biases")
        nc.vector.memset(biases[:, 0:1], 1e-6)
        nc.vector.memset(biases[:, 1:2], EPS)
        nc.vector.memset(biases[:, 2:3], 1e-6 * DH)

        cwt = []
        for ct in range(CT):
            t_ = persist.tile([128, 3 * CONV_K], fp32, name=f"cw{ct}",
                              tag=f"cw{ct}")
            nc.sync.dma_start(t_[:], cw_t[ct])
            cwt.append(t_)

        wo_s = []
        for ct in range(CT):
            t_ = wop.tile([128, D], bf16, name=f"wo{ct}", tag="wo")
            nc.sync.dma_start(t_[:], wo_t[ct])
            wo_s.append(t_)

        qh, kh, vh = [], [], []
        for lst, nm in ((qh, "q"), (kh, "k"), (vh, "v")):
            for ct in range(CT):
                lst.append(qkvp.tile([128, T], bf16, name=f"{nm}hat{ct}",
                                     tag="qkv"))

        # ================= phase A: projections + conv + silu + l2norm ====
        with tc.tile_pool(name="xp", bufs=KT) as xp, \
             tc.tile_pool(name="wp", bufs=3 * KT) as wp, \
             tc.tile_pool(name="rawp", bufs=4) as rawp, \
             tc.tile_pool(name="sqp", bufs=4) as sqp, \
             tc.tile_pool(name="bcp", bufs=3) as bcp:

            xt = []
            for kt in range(KT):
                t_ = xp.tile([128, T], bf16, name=f"xt{kt}", tag="xt")
                nc.sync.dma_start(t_[:], xT_t[kt])
                xt.append(t_)
            ws = {}
            for nm in ("q", "k", "v"):
                ws[nm] = []
                for kt in range(KT):
                    t_ = wp.tile([128, CG], bf16, name=f"w{nm}{kt}", tag="w")
                    nc.sync.dma_start(t_[:], w_t[nm][kt])
                    ws[nm].append(t_)

            for ti, (nm, dest) in enumerate((("q", qh), ("k", kh), ("v", vh))):
                sq_tiles = []
                for ct in range(CT):
                    rawt = rawp.tile([128, NT], bf16, name=f"raw{nm}{ct}",
                                     tag="raw")
                    nc.vector.memset(rawt[:, 0:PAD], 0.0)
                    dst = dest[ct]
                    for nb in range(NB):
                        pt = psw.tile([128, TOKB], fp32, name=f"pp{nm}{ct}{nb}",
                                      tag="w")
                        for kt in range(KT):
                            nc.tensor.matmul(
                                pt[:], ws[nm][kt][:, ct * 128:(ct + 1) * 128],
                                xt[kt][:, nb * TOKB:(nb + 1) * TOKB],
                                start=(kt == 0), stop=(kt == KT - 1))
                        nc.scalar.copy(
                            rawt[:, PAD + nb * TOKB:PAD + (nb + 1) * TOKB],
                            pt[:])
                    # causal depthwise conv along t
                    w0 = cwt[ct][:, ti * CONV_K:ti * CONV_K + 1]
                    nc.vector.tensor_scalar_mul(dst[:], rawt[:, 1:1 + T], w0)
                    for i in range(1, CONV_K):
                        wi = cwt[ct][:, ti * CONV_K + i:ti * CONV_K + i + 1]
                        nc.vector.scalar_tensor_tensor(
                            dst[:], rawt[:, 1 + i:1 + i + T], wi, dst[:],
                            ALU.mult, ALU.add)
                    if SILU_NATIVE:
                        nc.scalar.activation(dst[:], dst[:], AF.Silu)
                    else:
                        sg = rawp.tile([128, T], bf16, name=f"sg{nm}{ct}",
                                       tag="raw")
                        nc.scalar.activation(sg[:], dst[:], AF.Sigmoid)
                        nc.vector.tensor_mul(dst[:], dst[:], sg[:])
                    if ti < 2:
                        sqt = sqp.tile([128, T], bf16, name=f"sq{nm}{ct}",
                                       tag="sq")
                        nc.scalar.activation(sqt[:], dst[:], AF.Square)
                        sq_tiles.append(sqt)
                if ti < 2:
                    # per-head l2norm: ones-matrix matmul gives the per-token
                    # sum of squares broadcast to all 128 partitions at once;
                    # rsq = scale/sqrt(ss + 1e-6) applied per token block.
                    for head in range(2):
                        for nb in range(NB):
                            bc = psw.tile([128, TOKB], fp32,
                                          name=f"bc{nm}{head}{nb}", tag="w")
                            for cth in range(2):
                                nc.tensor.matmul(
                                    bc[:], ones128,
                                    sq_tiles[head * 2 + cth][
                                        :, nb * TOKB:(nb + 1) * TOKB],
                                    start=(cth == 0), stop=(cth == 1))
                            bcf = bcp.tile([128, TOKB], fp32,
                                           name=f"bcf{nm}{head}{nb}",
                                           tag="bcf")
                            if ti == 0:
                                # fold Dh^-0.5: 1/(16 sqrt(ss+eps)) =
                                # 1/sqrt(256 ss + 256 eps)
                                nc.scalar.activation(bcf[:], bc[:], AF.Sqrt,
                                                     bias=biases[:, 2:3],
                                                     scale=float(DH))
                            else:
                                nc.scalar.activation(bcf[:], bc[:], AF.Sqrt,
                                                     bias=biases[:, 0:1])
                            nc.vector.reciprocal(bcf[:], bcf[:])
                            bcb = bcp.tile([128, TOKB], bf16,
                                           name=f"bcb{nm}{head}{nb}",
                                           tag="bcb")
                            nc.scalar.copy(bcb[:], bcf[:])
                            sl = slice(nb * TOKB, (nb + 1) * TOKB)
                            for cth in range(2):
                                ct = head * 2 + cth
                                nc.vector.tensor_mul(dest[ct][:, sl],
                                                     dest[ct][:, sl], bcb[:])

        # ====== phase B + C: delta-rule recurrence, heads interleaved =====
        with tc.tile_pool(name="recp", bufs=1) as recp, \
             tc.tile_pool(name="otp", bufs=4) as otp, \
             tc.tile_pool(name="ofp", bufs=4) as ofp:
            s_ps, s_sb = [], []
            for head in range(2):
                s_ps.append(pss.tile([128, 512], fp32, name=f"sps{head}",
                                     tag="sps"))
                t_ = recp.tile([128, 512], bf16, name=f"ssb{head}", tag="ssb",
                               bufs=4)
                nc.vector.memset(t_[:], 0.0)
                s_sb.append(t_)
            oTc = [None, None]

            def chunk_head(ch, head):
                """Generator emitting one head's chunk ops; yields between
                pipeline stages so the two heads' streams interleave."""
                t0 = ch * C
                ct0 = head * 2
                QT = [qh[ct0][:, t0:t0 + C], qh[ct0 + 1][:, t0:t0 + C]]
                KTt = [kh[ct0][:, t0:t0 + C], kh[ct0 + 1][:, t0:t0 + C]]
                VT = [vh[ct0][:, t0:t0 + C], vh[ct0 + 1][:, t0:t0 + C]]

                # K, V in [C, Dh] layout via PE transpose (bf16 psum)
                ptkv = psw.tile([128, 512], bf16, name=f"ptkv{head}{ch}",
                                tag="w")
                for i in range(2):
                    nc.tensor.transpose(ptkv[:, i * 128:(i + 1) * 128],
                                        KTt[i], ident)
                    nc.tensor.transpose(
                        ptkv[:, 256 + i * 128:256 + (i + 1) * 128],
                        VT[i], ident)
                kvcd = recp.tile([128, 512], bf16, name=f"kvcd{head}{ch}",
                                 tag="kvcd", bufs=4)
                nc.vector.tensor_copy(kvcd[:], ptkv[:])
                yield

                # KK^T / KQ^T share one psum bank; masked pieces in SBUF bf16
                pkx = psw.tile([128, 256], fp32, name=f"pkx{head}{ch}",
                               tag="w")
                pkk, pkq = pkx[:, 0:128], pkx[:, 128:256]
                for i in range(2):
                    nc.tensor.matmul(pkk, KTt[i], KTt[i], start=(i == 0),
                                     stop=(i == 1))
                for i in range(2):
                    nc.tensor.matmul(pkq, KTt[i], QT[i], start=(i == 0),
                                     stop=(i == 1))
                pkkS = recp.tile([128, 128], bf16, name=f"pkkS{head}{ch}",
                                 tag="pkkS", bufs=4)
                nc.scalar.copy(pkkS[:], pkk)
                pkqS = recp.tile([128, 128], bf16, name=f"pkqS{head}{ch}",
                                 tag="pkqS", bufs=4)
                nc.scalar.copy(pkqS[:], pkq)
                yield

                Nl = recp.tile([128, 128], bf16, name=f"Nl{head}{ch}",
                               tag="Nl", bufs=4)
                Nu = recp.tile([128, 128], bf16, name=f"Nu{head}{ch}",
                               tag="Nu", bufs=4)
                R0 = recp.tile([128, 128], bf16, name=f"R0{head}{ch}",
                               tag="R0", bufs=4)
                Pat = recp.tile([128, 128], bf16, name=f"Pat{head}{ch}",
                                tag="Pat", bufs=4)
                nc.vector.tensor_mul(Nl[:], pkkS[:], m_bdl)
                nc.vector.tensor_mul(Nu[:], pkkS[:], m_bdu)
                nc.vector.tensor_sub(R0[:], ident, Nu[:])
                nc.vector.tensor_mul(Pat[:], pkqS[:], m_triuI)
                yield

                # R ~= (I+B)^{-1} = sum_{k<32} (-B)^k via Neumann doubling
                # (full 128x128 strict-triangular B; truncation err ~1e-5)
                pR = psw.tile([128, 128], fp32, name=f"pR{head}{ch}",
                              tag="w")
                nc.tensor.matmul(pR[:], ident, R0[:], start=True, stop=True)
                Rm, Pm, Qm = R0, Nl, Nu
                for lvl in range(4):
                    ppq = psw.tile([128, 256], fp32,
                                   name=f"ppq{head}{ch}{lvl}", tag="w")
                    nc.tensor.matmul(ppq[:, 0:128], Qm[:], Pm[:],
                                     start=True, stop=True)
                    Pn = recp.tile([128, 128], bf16,
                                   name=f"Pn{head}{ch}{lvl}", tag="Pn",
                                   bufs=5)
                    nc.vector.tensor_copy(Pn[:], ppq[:, 0:128])
                    if lvl < 3:
                        nc.tensor.matmul(ppq[:, 128:256], Pm[:], Qm[:],
                                         start=True, stop=True)
                        Qn = recp.tile([128, 128], bf16,
                                       name=f"Qn{head}{ch}{lvl}", tag="Qn",
                                       bufs=4)
                        nc.scalar.copy(Qn[:], ppq[:, 128:256])
                    else:
                        Qn = None
                    nc.tensor.matmul(pR[:], Pn[:], Rm[:], start=False,
                                     stop=True, skip_group_check=True)
                    Rn = recp.tile([128, 128], bf16,
                                   name=f"Rm{head}{ch}{lvl}", tag="Rm",
                                   bufs=5)
                    nc.scalar.copy(Rn[:], pR[:])
                    Rm, Pm, Qm = Rn, Pn, Qn
                    yield

                # RHS' = V - K S    (psum = K@S, then V - psum on DVE)
                pks = psw.tile([128, 256], fp32, name=f"pks{head}{ch}",
                               tag="w")
                for i in range(2):
                    nc.tensor.matmul(pks[:], KTt[i],
                                     s_sb[head][:, i * 256:(i + 1) * 256],
                                     start=(i == 0), stop=(i == 1))
                rhs_sb = recp.tile([128, 256], bf16, name=f"rhs{head}{ch}",
                                   tag="rhs", bufs=4)
                nc.vector.tensor_sub(rhs_sb[:], kvcd[:, 256:512], pks[:])
                yield

                # U' = R^T RHS'  (one matmul, no forward substitution)
                pu = psw.tile([128, 256], fp32, name=f"pu{head}{ch}",
                              tag="w")
                nc.tensor.matmul(pu[:], Rm[:], rhs_sb[:], start=True,
                                 stop=True)
                u_sb = recp.tile([128, 256], bf16, name=f"u{head}{ch}",
                                 tag="u", bufs=4)
                nc.vector.tensor_copy(u_sb[:], pu[:])
                yield

                # O = Q S + P^T U'
                po = psw.tile([128, 256], fp32, name=f"po{head}{ch}",
                              tag="w")
                for i in range(2):
                    nc.tensor.matmul(po[:], QT[i],
                                     s_sb[head][:, i * 256:(i + 1) * 256],
                                     start=(i == 0), stop=False)
                nc.tensor.matmul(po[:], Pat[:], u_sb[:], start=False,
                                 stop=True)

                # S += K^T U'   (accumulate in persistent psum)
                for i in range(2):
                    nc.tensor.matmul(s_ps[head][:, i * 256:(i + 1) * 256],
                                     kvcd[:, i * 128:(i + 1) * 128],
                                     u_sb[:],
                                     start=(ch == 0 and i == 0), stop=True,
                                     skip_group_check=True)
                s_nb = recp.tile([128, 512], bf16, name=f"ssb{head}{ch}",
                                 tag="ssb", bufs=4)
                nc.scalar.copy(s_nb[:, 0:256], s_ps[head][:, 0:256])
                nc.vector.tensor_copy(s_nb[:, 256:512],
                                      s_ps[head][:, 256:512])
                s_sb[head] = s_nb
                yield

                # RMSNorm rows of O, then transpose out
                osq = recp.tile([128, 256], bf16, name=f"osq{head}{ch}",
                                tag="osq", bufs=4)
                ossq = recp.tile([128, 1], fp32, name=f"ossq{head}{ch}",
                                 tag="ossq", bufs=4)
                nc.scalar.activation(osq[:], po[:], AF.Square,
                                     accum_out=ossq[:])
                orsq = recp.tile([128, 1], fp32, name=f"orsq{head}{ch}",
                                 tag="orsq", bufs=4)
                nc.scalar.activation(orsq[:], ossq[:], AF.Sqrt,
                                     bias=biases[:, 1:2], scale=1.0 / DH)
                nc.vector.reciprocal(orsq[:], orsq[:])
                onrm = recp.tile([128, 256], bf16, name=f"onrm{head}{ch}",
                                 tag="onrm", bufs=4)
                nc.vector.tensor_scalar_mul(onrm[:], po[:], orsq[:])
                yield

                pto = psw.tile([128, 256], bf16, name=f"pto{head}{ch}",
                               tag="w")
                for i in range(2):
                    nc.tensor.transpose(pto[:, i * 128:(i + 1) * 128],
                                        onrm[:, i * 128:(i + 1) * 128],
                                        ident)
                oTt = otp.tile([128, 256], bf16, name=f"oT{head}{ch}",
                               tag="oT")
                nc.vector.tensor_copy(oTt[:], pto[:])
                oTc[head] = oTt

            for ch in range(NCHUNK):
                gens = [chunk_head(ch, 0), chunk_head(ch, 1)]
                while gens:
                    for g in list(gens):
                        try:
                            next(g)
                        except StopIteration:
                            gens.remove(g)

                # ---- output projection for this chunk (both heads) ----
                if DEBUG_SKIP_WO:
                    continue
                for half in range(2):
                    pf = psw.tile([128, 512], fp32, name=f"pf{ch}{half}",
                                  tag="w")
                    k = 0
                    for hd in range(2):
                        for i in range(2):
                            nc.tensor.matmul(
                                pf[:], oTc[hd][:, i * 128:(i + 1) * 128],
                                wo_s[hd * 2 + i][:, half * 512:(half + 1) * 512],
                                start=(k == 0), stop=(k == 3))
                            k += 1
                    of = ofp.tile([128, 512], fp32, name=f"of{ch}{half}",
                                  tag="of")
                    if half == 0:
                        nc.scalar.copy(of[:], pf[:])
                    else:
                        nc.vector.tensor_copy(of[:], pf[:])
                    nc.sync.dma_start(
                        out_t[ch][:, half * 512:(half + 1) * 512], of[:])


LP_NP = np.float16  # host-side 16-bit dtype matching the device dtype


def _make_consts():
    ii = np.arange(128)
    ident = np.eye(128, dtype=np.float32)
    bdl = (ii[:, None] > ii[None, :]).astype(np.float32)
    bdu = (ii[:, None] < ii[None, :]).astype(np.float32)
    fneg = np.zeros((128, 128), np.float32)
    triuI = (ii[:, None] <= ii[None, :]).astype(np.float32)
    ones = np.ones((128, 128), np.float32)
    return np.concatenate([ident, bdl, bdu, fneg, triuI, ones],
                          axis=1).astype(LP_NP)


def _get_compiled():
    key = ("nc", SILU_NATIVE)
    if key not in _CACHE:
        _CACHE[key] = _build_bass()
    return _CACHE[key]


def kernel(hidden_states, Wq, Wk, Wv, conv_wq, conv_wk, conv_wv, onorm_w, Wo):
    from concourse.bass_utils import run_bass_kernel_spmd

    hidden_states = np.asarray(hidden_states, np.float32)
    Wq = np.asarray(Wq, np.float32)
    Wk = np.asarray(Wk, np.float32)
    Wv = np.asarray(Wv, np.float32)
    Wo = np.asarray(Wo, np.float32)
    conv_wq = np.asarray(conv_wq, np.float32)
    conv_wk = np.asarray(conv_wk, np.float32)
    conv_wv = np.asarray(conv_wv, np.float32)
    onorm_w = np.asarray(onorm_w, np.float32)

    bf = LP_NP
    consts = _make_consts()
    Wo_eff = (Wo * np.tile(onorm_w, H)[:, None]).astype(bf)  # fold RMS weight

    in_maps = []
    for core in range(NCORES):
        b, g = divmod(core, 2)
        cols = slice(CG * g, CG * (g + 1))
        in_maps.append({
            "xT": np.ascontiguousarray(hidden_states[b].T).astype(bf),
            "wq": np.ascontiguousarray(Wq[:, cols]).astype(bf),
            "wk": np.ascontiguousarray(Wk[:, cols]).astype(bf),
            "wv": np.ascontiguousarray(Wv[:, cols]).astype(bf),
            "wo": np.ascontiguousarray(Wo_eff[cols, :]),
            "cw": np.ascontiguousarray(np.concatenate(
                [conv_wq[cols], conv_wk[cols], conv_wv[cols]], axis=1)),
            "consts": consts,
        })

    nc = _get_compiled()
    res = run_bass_kernel_spmd(nc, in_maps, core_ids=list(range(NCORES)),
                               **_CACHE.get("run_kwargs", {}))
    _CACHE["last_results"] = res
    out = np.zeros((B, T, D), np.float32)
    for core in range(NCORES):
        out[core // 2] += res.results[core]["out"]
    return out
